# revision 1
# baseline (speedup 1.0000x reference)
"""BottleneckAdapter kernel for Trainium2 (Bass/Tile), 8-way data parallel.

out = x + scale * (gelu(LN(x) @ w_down + b_down) @ w_up + b_up)

Strategy per core (2048 tokens of the 16384 total, weights replicated):
  - Stats via DVE bn_stats/bn_aggr straight from the fp32 input (one pass
    -> mean+var per token), so the cast can happen AFTER the stats and the
    whole LayerNorm normalize folds into ONE ACT pass per tile:
    xbn = Identity(x * rstd + (-mu*rstd)) with per-partition scale/bias
    operands, cast to bf16 on the way out. norm_w is folded into the down
    weights, (b_down + norm_b @ w_down) into the gelu bias.
  - The down-matmul needs x with D on partitions. The transpose runs on
    the tensor engine (8 is_transpose matmuls per 128-token tile into a
    PSUM bank, evacuated by bf16 2x-packed copies) so DMA carries ONLY
    the pure HBM traffic (8 MB in + 8 MB out per core ~ 47us at
    358 GB/s), fully pipelined: HWDGE loads start immediately, stores
    stream out per half-quarter as soon as each is computed.
  - Down-proj is computed transposed, two tiles at a time:
    zT[64, 256] = W'^T @ xbn^T (W' chunks stationary, xbn pairs moving),
    and gelu reads zT straight from PSUM with the bias as a per-partition
    operand, writing the [65, 256] up-proj operand (row 64 is a preset
    ones row applying the scaled up-bias).
  - The residual is exact fp32. Per-tile work assignment across ACT/DVE/
    GpSimd is tuned so no engine exceeds the HBM roofline: DVE does
    bn_stats, some castnorms, some PSUM evacuations and most residual
    adds; ACT does most castnorms, most xT evacuations and the PSUM
    evacuation for GpSimd-assisted tiles; GpSimd does a few SBUF-only
    residual adds plus the store DMAs.
"""

import numpy as np

import concourse.bass as bass
import concourse.bacc as bacc
import concourse.mybir as mybir
import concourse.tile as tile
from concourse import bass_utils
from concourse.masks import make_identity

F32 = mybir.dt.float32
F16 = mybir.dt.float16
BF16 = mybir.dt.bfloat16
AF = mybir.ActivationFunctionType
OP = mybir.AluOpType

# Problem shapes (hardcoded per the contract).
B, N, D = 4, 4096, 1024
BN = 64                      # bottleneck
N_CORES = 8
TOK_TOTAL = B * N            # 16384
TOK = TOK_TOTAL // N_CORES   # 2048 tokens per core
P = 128                      # partitions
NT = TOK // P                # 16 token tiles per core
NQ = 4                       # quarters (load/store granularity)
TPQ = NT // NQ               # 4 token tiles per quarter
NCH = D // P                 # 8 contraction chunks of 128
EPS = 1e-5
H = D // 2                   # 512 (psum bank width)

# Per-tile engine assignment (global tile index 0..15), tuned from traces.
CN_DVE = {2, 6, 10, 14, 3, 7}          # castnorm on DVE (else ACT)
EV_DVE = {1, 5, 9, 13, 15}             # xT evacuation on DVE (else ACT)
RS_GPS = {0, 1, 4, 5, 8, 9}            # residual via ACT-evac + GpSimd add


def _build_kernel():
    nc = bacc.Bacc(
        "TRN2",
        target_bir_lowering=False,
        debug=False,
        enable_asserts=False,
        num_devices=N_CORES,
    )
    x_d = nc.dram_tensor("x", [TOK, D], F32, kind="ExternalInput")
    nw_d = nc.dram_tensor("norm_w", [D], F32, kind="ExternalInput")
    nb_d = nc.dram_tensor("norm_b", [D], F32, kind="ExternalInput")
    wd_d = nc.dram_tensor("w_down", [D, BN], F32, kind="ExternalInput")
    bd_d = nc.dram_tensor("b_down", [BN], F32, kind="ExternalInput")
    wu_d = nc.dram_tensor("w_up", [BN, D], F32, kind="ExternalInput")
    bu_d = nc.dram_tensor("b_up", [D], F32, kind="ExternalInput")
    sc_d = nc.dram_tensor("scale", [1, 1], F32, kind="ExternalInput")
    out_d = nc.dram_tensor("out", [TOK, D], F32, kind="ExternalOutput")

    with tile.TileContext(nc) as tc:
        _body(
            tc,
            x_d.ap(),
            nw_d.ap(),
            nb_d.ap(),
            wd_d.ap(),
            bd_d.ap(),
            wu_d.ap(),
            bu_d.ap(),
            sc_d.ap(),
            out_d.ap(),
        )
    nc.compile()
    return nc


def _body(tc, x, nw, nb, wd, bd, wu, bu, sc, out):
    from contextlib import ExitStack

    nc = tc.nc
    ctx = ExitStack()
    with ctx:
        x_r = x.rearrange("(t p) d -> p t d", p=P)      # [128, 16, 1024]
        out_r = out.rearrange("(t p) d -> p t d", p=P)

        const = ctx.enter_context(tc.tile_pool(name="const", bufs=1))
        px = ctx.enter_context(tc.tile_pool(name="px", bufs=4))       # x f32 quarters

        # ---------- tiny const loads first (<1us total on the HWDGE FIFO),
        # then the 8 MB of x loads; preproc inputs land immediately.
        w_f32 = const.tile([P, NCH, BN], F32)
        nc.sync.dma_start(out=w_f32, in_=wd.rearrange("(c p) j -> p c j", p=P))
        nw_sb = const.tile([P, NCH], F32)
        nc.sync.dma_start(out=nw_sb, in_=nw.rearrange("(c p) -> p c", p=P))

        xqs = []
        for q in range(NQ):
            xq = px.tile([P, TPQ, D], F32, tag="xq")
            hq = TPQ // 2
            nc.sync.dma_start(
                out=xq[:, 0:hq, :], in_=x_r[:, q * TPQ : q * TPQ + hq, :]
            )
            nc.sync.dma_start(
                out=xq[:, hq:TPQ, :], in_=x_r[:, q * TPQ + hq : (q + 1) * TPQ, :]
            )
            xqs.append(xq)

        # ---------- constants / preprocessing (no gpsimd element-wise) ----
        eps_b = const.tile([P, 1], F32)
        nc.vector.memset(eps_b, EPS)

        # W' = norm_w[:,None] * w_down laid out [p, c, j]; bf16.
        w_sb = const.tile([P, NCH, BN], BF16)
        for c in range(NCH):
            nc.vector.tensor_scalar_mul(
                w_sb[:, c, :], w_f32[:, c, :], nw_sb[:, c : c + 1]
            )

        ident_bf = const.tile([P, P], BF16)
        make_identity(nc, ident_bf)

        # norm_b laid out [p, c] for the b' matvec; b_down as a column.
        nb_sb = const.tile([P, NCH, 1], F32)
        nc.sync.dma_start(out=nb_sb[:, :, 0], in_=nb.rearrange("(c p) -> p c", p=P))
        bd_col = const.tile([BN, 1], F32)
        nc.sync.dma_start(out=bd_col, in_=bd[:, None])

        # w_up_ext = scale * [w_up; b_up]  -> bf16 [65, 1024]
        wue_f = const.tile([BN + 1, D], F32)
        nc.sync.dma_start(out=wue_f[0:BN, :], in_=wu)
        nc.sync.dma_start(out=wue_f[BN : BN + 1, :], in_=bu[None, :])
        sc_b = const.tile([BN + 1, 1], F32)
        nc.gpsimd.dma_start(
            out=sc_b,
            in_=bass.AP(tensor=sc.tensor, offset=0, ap=[[0, BN + 1], [1, 1]]),
        )
        wue = const.tile([BN + 1, D], BF16)
        nc.vector.tensor_scalar_mul(wue, wue_f, sc_b)

        # ---------- pools ----------
        pxn = ctx.enter_context(tc.tile_pool(name="pxn", bufs=6))     # normalized bf16
        pxt = ctx.enter_context(tc.tile_pool(name="pxt", bufs=3))     # xT pairs sbuf
        pst = ctx.enter_context(tc.tile_pool(name="pst", bufs=8))     # per-quarter stats
        pbs = ctx.enter_context(tc.tile_pool(name="pbs", bufs=4))     # bn_stats scratch
        pgt = ctx.enter_context(tc.tile_pool(name="pgt", bufs=4))     # gT pair tiles
        phs = ctx.enter_context(tc.tile_pool(name="phs", bufs=2))     # u evac staging
        pout = ctx.enter_context(tc.tile_pool(name="pout", bufs=3))   # out staging
        xtps = ctx.enter_context(tc.tile_pool(name="xtps", bufs=2, space="PSUM"))
        zps = ctx.enter_context(tc.tile_pool(name="zps", bufs=2, space="PSUM"))
        ups = ctx.enter_context(tc.tile_pool(name="ups", bufs=2, space="PSUM"))

        # b' column: b_down + norm_b @ w_down  -> [64, 1] (gelu bias operand)
        bp_ps = zps.tile([BN, 2 * P], F32, tag="zt")
        for c in range(NCH):
            nc.tensor.matmul(
                bp_ps[:, 0:1], w_f32[:, c, :], nb_sb[:, c, :],
                start=(c == 0), stop=(c == NCH - 1),
            )
        b_col = const.tile([BN, 1], F32)
        nc.vector.scalar_tensor_tensor(
            out=b_col, in0=bp_ps[:, 0:1], scalar=1.0, in1=bd_col,
            op0=OP.mult, op1=OP.add,
        )

        # gelu output tiles are allocated once: row BN (the ones row feeding
        # the bias term of the up-projection) is preset a single time.
        gts = []
        for _ in range(4):
            gt = pgt.tile([BN + 1, 2 * P], BF16, tag="gt")
            nc.vector.memset(gt[BN : BN + 1, :], 1.0)
            gts.append(gt)

        state = {}

        def phase_s(q):
            """bn_stats/bn_aggr per tile + per-quarter rstd/-mu*rstd."""
            xq = xqs[q]
            mv = pst.tile([P, TPQ, 2], F32, tag="mv")
            for i in range(TPQ):
                bns = pbs.tile([P, 2, 6], F32, tag="bns")
                nc.vector.bn_stats(bns[:, 0, :], xq[:, i, 0:H])
                nc.vector.bn_stats(bns[:, 1, :], xq[:, i, H:D])
                nc.vector.bn_aggr(mv[:, i, :], bns)
            srt = pst.tile([P, TPQ], F32, tag="srt")
            nc.scalar.activation(srt, mv[:, :, 1], AF.Sqrt, bias=eps_b)
            rstd_q = pst.tile([P, TPQ], F32, tag="rstd")
            nc.vector.reciprocal(rstd_q, srt)
            nmr_q = pst.tile([P, TPQ], F32, tag="nmr")
            nc.vector.scalar_tensor_tensor(
                out=nmr_q, in0=mv[:, :, 0], scalar=-1.0, in1=rstd_q,
                op0=OP.mult, op1=OP.mult,
            )
            state[q] = (rstd_q, nmr_q)

        def phase_n(q):
            """fused cast+normalize, PE transposes, PSUM evacuation."""
            xq = xqs[q]
            rstd_q, nmr_q = state[q]
            xt_pairs = []
            for pair in range(TPQ // 2):
                xt_pair = pxt.tile([P, 2, D], BF16, tag="xts")
                for j in range(2):
                    i = pair * 2 + j
                    g = q * TPQ + i
                    xbn = pxn.tile([P, D], BF16, tag="xbn")
                    if g in CN_DVE:
                        nc.vector.tensor_scalar(
                            out=xbn, in0=xq[:, i, :],
                            scalar1=rstd_q[:, i : i + 1],
                            scalar2=nmr_q[:, i : i + 1],
                            op0=OP.mult, op1=OP.add,
                        )
                    else:
                        nc.scalar.activation(
                            xbn, xq[:, i, :], AF.Identity,
                            scale=rstd_q[:, i : i + 1],
                            bias=nmr_q[:, i : i + 1],
                        )
                    xt_ps = xtps.tile([P, D], BF16, tag="xt")
                    for c in range(NCH):
                        nc.tensor.transpose(
                            xt_ps[:, c * P : (c + 1) * P],
                            xbn[:, c * P : (c + 1) * P],
                            ident_bf,
                        )
                    if g in EV_DVE:
                        nc.vector.tensor_copy(xt_pair[:, j, :], xt_ps)
                    else:
                        nc.scalar.copy(xt_pair[:, j, :], xt_ps)
                xt_pairs.append(xt_pair)
            state[q] = xt_pairs

        def phase_b(q):
            """down (zT pairs) + gelu + up + residual + store for quarter q."""
            xq = xqs[q]
            xt_pairs = state.pop(q)
            ot = pout.tile([P, TPQ, D], F32, tag="ot")
            for pair in range(TPQ // 2):
                i0 = pair * 2
                zt = zps.tile([BN, 2 * P], F32, tag="zt")
                for c in range(NCH):
                    nc.tensor.matmul(
                        zt,
                        w_sb[:, c, :],
                        xt_pairs[pair][:, :, c * P : (c + 1) * P],
                        start=(c == 0),
                        stop=(c == NCH - 1),
                    )
                gt = gts[(q * 2 + pair) % 4]
                nc.scalar.activation(gt[0:BN, :], zt, AF.Gelu, bias=b_col)
                for j in range(2):
                    i = i0 + j
                    g = q * TPQ + i
                    u = ups.tile([P, D], F32, tag="u")
                    for h in range(2):
                        nc.tensor.matmul(
                            u[:, h * H : (h + 1) * H],
                            gt[:, j * P : (j + 1) * P],
                            wue[:, h * H : (h + 1) * H],
                            start=True,
                            stop=True,
                        )
                    if g in RS_GPS:
                        # ACT evacuates PSUM; GpSimd (SBUF-only) does the add.
                        hs = phs.tile([P, D], F32, tag="hs")
                        nc.scalar.copy(hs, u)
                        nc.gpsimd.tensor_add(ot[:, i, :], hs, xq[:, i, :])
                    else:
                        # DVE: out = u + x straight from PSUM (fp32).
                        nc.vector.tensor_add(ot[:, i, :], u, xq[:, i, :])
            hq = TPQ // 2
            nc.gpsimd.dma_start(
                out=out_r[:, q * TPQ : q * TPQ + hq, :], in_=ot[:, 0:hq, :]
            )
            nc.gpsimd.dma_start(
                out=out_r[:, q * TPQ + hq : (q + 1) * TPQ, :], in_=ot[:, hq:TPQ, :]
            )

        # strict per-quarter pipeline order: engine queues are FIFO, so any
        # later-quarter work emitted early blocks this quarter's tail ops.
        for q in range(NQ):
            phase_s(q)
            phase_n(q)
            phase_b(q)


_NC = None


def _get_nc():
    global _NC
    if _NC is None:
        _NC = _build_kernel()
    return _NC


def _make_in_maps(inputs):
    x = np.ascontiguousarray(np.asarray(inputs["x"], dtype=np.float32)).reshape(
        TOK_TOTAL, D
    )
    shared = {
        "norm_w": np.ascontiguousarray(np.asarray(inputs["norm_w"], np.float32)),
        "norm_b": np.ascontiguousarray(np.asarray(inputs["norm_b"], np.float32)),
        "w_down": np.ascontiguousarray(np.asarray(inputs["w_down"], np.float32)),
        "b_down": np.ascontiguousarray(np.asarray(inputs["b_down"], np.float32)),
        "w_up": np.ascontiguousarray(np.asarray(inputs["w_up"], np.float32)),
        "b_up": np.ascontiguousarray(np.asarray(inputs["b_up"], np.float32)),
        "scale": np.asarray(inputs["scale"], np.float32).reshape(1, 1),
    }
    in_maps = []
    for c in range(N_CORES):
        m = dict(shared)
        m["x"] = np.ascontiguousarray(x[c * TOK : (c + 1) * TOK])
        in_maps.append(m)
    return in_maps


def run(inputs, trace=False, **kwargs):
    nc = _get_nc()
    in_maps = _make_in_maps(inputs)
    res = bass_utils.run_bass_kernel_spmd(
        nc, in_maps, core_ids=list(range(N_CORES)), trace=trace, **kwargs
    )
    shards = [res.results[c]["out"] for c in range(N_CORES)]
    full = np.concatenate(shards, axis=0).reshape(B, N, D).astype(np.float32)
    return full, res


def kernel(**inputs):
    full, _ = run(inputs, trace=False)
    return full

